# revision 32
# baseline (speedup 1.0000x reference)
"""MoE block (B=16, C=192, H=W=32, E=8, top-2, 3x3 same-conv experts) on 8 trn2 cores.

Strategy (v6, compact flat-run pixel-stationary):
  - Router + top-2 combine computed on host; conv linearity folds the
    expert mix into ONE conv per sample (combined weights). 2 convs/core.
  - Matmul formulation: out[M=pixels, N=192 out-ch], pixel positions
    STATIONARY, weights MOVING. Streamed rows per matmul = 192 (vs 512
    for the out-ch-stationary form).
  - Walrus requires the stationary AP to have a single free dimension,
    so the image is stored 32-wide (no column padding, only top/bottom
    pad rows): real outputs are flat [32, 1056) = exactly 8 contiguous
    128-runs, and a 3x3 tap is a uniform flat shift s = (dy-1)*32+(dx-1)
    of the run. Horizontal taps wrap across row boundaries, corrupting
    output columns 0 and 31 — the host recomputes those two columns
    exactly (tiny einsum) and overwrites them.
  - Contraction (9 taps x 192 ch = 1728) in 14 K-chunks per run: 9 full
    chunks for ch 0-127 (one per tap, plain image tile Ta), and
    ch 128-191 packed two-taps-per-chunk using duplicate tiles whose
    upper 64 partitions hold the same channels pre-shifted by the
    inter-tap flat offset (d=32 vertical pairs, d=1 horizontal), so one
    AP covers both tap halves: 4 paired chunks + 1 single K=64 chunk.
  - 2 samples x 8 runs x 14 chunks = 224 matmuls of 192 rows — the
    128x128 PE packing floor for this conv in bf16.
  - PE kept continuously busy from ~1.1us with warmup matmuls (an idle
    PE gap resets the p-state ramp).
  - Input DMAs on SP (HWDGE) + Pool (SWDGE) in consumption order;
    output DMAs on Pool; the DMA-completion semaphore fires ~900ns
    after the transfer, so everything is prefetched one phase ahead.
"""

import numpy as np

B, C, H, W = 16, 192, 32, 32
E, TOPK = 8, 2
NCORES = 8
S = B // NCORES          # samples per core
RW = 32                  # row width (no column padding)
FP = 34 * RW             # flat image size 1088 (rows 0/33 are pad rows)
MARG = 33                # flat-shift margin
TF = FP + 2 * MARG       # tile free size 1154
NR = 8                   # M-runs per sample (8 x 128)
RUNS = [(RW + 128 * i, 128) for i in range(NR)]
NCH = 14                 # K-chunks per run
TAPS = [(t // 3, t % 3) for t in range(9)]

# Warmup matmul row-counts: first runs at the LOW p-state, rest at MID.
WARMUP_NS = [192] * 14

_cache = {}


def _shift(t):
    dy, dx = TAPS[t]
    return (dy - 1) * RW + (dx - 1)


def _build_module():
    import concourse.tile as tile
    from concourse import bacc, mybir

    f32 = mybir.dt.float32
    bf16 = mybir.dt.bfloat16

    nc = bacc.Bacc("TRN2", target_bir_lowering=False, debug=False, num_devices=NCORES)
    ta_d = nc.dram_tensor("ta", [S, 128, TF + 4 * C], bf16, kind="ExternalInput")
    tb32_d = nc.dram_tensor("tb32", [S, 128, TF], bf16, kind="ExternalInput")
    tb1_d = nc.dram_tensor("tb1", [S, 128, TF], bf16, kind="ExternalInput")
    w_d = nc.dram_tensor("w", [S, 128, (NCH - 4) * C], bf16, kind="ExternalInput")
    out_d = nc.dram_tensor("out", [S, 128, NR, C], f32, kind="ExternalOutput")

    with tile.TileContext(nc) as tc:
        with (
            tc.tile_pool(name="img", bufs=1) as img,
            tc.tile_pool(name="win", bufs=1) as win,
            tc.tile_pool(name="cst", bufs=1) as cst,
            tc.tile_pool(name="ps", bufs=8, space="PSUM") as ps,
            tc.tile_pool(name="oev", bufs=4) as oev,
        ):
            Ta, T32, T1, Wt = {}, {}, {}, {}

            wt0 = win.tile([128, (NCH - 4) * C], bf16, name="W_0", tag="W_0")
            Wt[0] = wt0
            wt1 = win.tile([128, (NCH - 4) * C], bf16, name="W_1", tag="W_1")
            Wt[1] = wt1
            for s, nm in ((0, "Ta_0"), (1, "Ta_1")):
                Ta[s] = img.tile([128, TF + 4 * C], bf16, name=nm, tag=nm)
            for s, nm in ((0, "T32_0"), (1, "T32_1")):
                T32[s] = img.tile([128, TF], bf16, name=nm, tag=nm)
            for s, nm in ((0, "T1_0"), (1, "T1_1")):
                T1[s] = img.tile([128, TF], bf16, name=nm, tag=nm)

            # --- input DMAs: SP (HWDGE) + Pool (SWDGE), consumption order ---
            # First two transfers (w chunks 0-1 via Pool, Ta runs 0-2 prefix
            # via SP) use both DGE paths in parallel for the earliest start.
            # Ta tile layout: [w c0-c1 (2C) | ta flat (TF) | w c2-c3 (2C)]
            P1 = 2 * C + MARG + RW + 128 * 3 + MARG  # first bite: runs 0-2 + wc0c1
            P2 = P1 + 128 * 2                        # ... extended to runs 3-4
            P3 = 2 * C + TF                          # end of ta region
            nc.sync.dma_start(Ta[0][:, 0:P1], ta_d[0, :, 0:P1])
            nc.sync.dma_start(Ta[0][:, P1:P2], ta_d[0, :, P1:P2])
            nc.gpsimd.dma_start(wt0[:, 0 : 5 * C], w_d[0, :, 0 : 5 * C])
            nc.sync.dma_start(Ta[0][:, P3 : P3 + 2 * C], ta_d[0, :, P3 : P3 + 2 * C])
            nc.sync.dma_start(T32[0][:], tb32_d[0])
            nc.gpsimd.dma_start(Ta[0][:, P2:P3], ta_d[0, :, P2:P3])
            nc.sync.dma_start(wt0[:, 5 * C : 10 * C], w_d[0, :, 5 * C : 10 * C])
            nc.sync.dma_start(T1[0][:], tb1_d[0])
            nc.gpsimd.dma_start(Ta[1][:], ta_d[1])
            nc.sync.dma_start(wt1[:], w_d[1])
            nc.sync.dma_start(T32[1][:], tb32_d[1])
            nc.sync.dma_start(T1[1][:], tb1_d[1])

            # --- PSUM tiles (8 banks, cycled) -------------------------------
            psb = {}
            for s in range(S):
                for r in range(NR):
                    psb[(s, r)] = ps.tile([128, C], f32, name=f"ps_{s}_{r}",
                                          tag="ps")

            # --- PE warmup on zeros -----------------------------------------
            scr = cst.tile([128, C], bf16, name="scr", tag="scr")
            nc.vector.memset(scr[:], 0.0)
            for n in WARMUP_NS:
                nc.tensor.matmul(psb[(0, 0)][:, 0:n], scr[:, 0:128], scr[:, 0:n],
                                 start=True, stop=True, skip_group_check=True)

            # --- matmul emission --------------------------------------------
            def lhsT(s, r, c):
                base, ln = RUNS[r]
                if c < 9:  # ch 0-127, tap c
                    st = 2 * C + MARG + base + _shift(c)
                    return Ta[s][:, st : st + ln]
                if c < 12:  # pairs (0,j)+(1,j), d=32 baked into T32's upper
                    st = MARG + base + _shift(c - 9)
                    return T32[s][:, st : st + ln]
                if c == 12:  # pair (2,0)+(2,1), d=1 baked into T1's upper
                    st = MARG + base + _shift(6)
                    return T1[s][:, st : st + ln]
                # single (2,2), K=64 on T1's lower half
                st = MARG + base + _shift(8)
                return T1[s][0:64, st : st + ln]

            def rhs(s, c):
                if c < 2:      # w c0-c1: head of the Ta tile
                    return Ta[s][:, c * C : (c + 1) * C]
                if c < 4:      # w c2-c3: tail of the Ta tile
                    return Ta[s][:, 2 * C + TF + (c - 2) * C : 2 * C + TF + (c - 1) * C]
                if c == NCH - 1:
                    return Wt[s][0:64, (c - 4) * C : (c - 3) * C]
                return Wt[s][:, (c - 4) * C : (c - 3) * C]

            def mm(s, r, c):
                nc.tensor.matmul(psb[(s, r)][:], lhsT(s, r, c), rhs(s, c),
                                 start=(c == 0), stop=(c == NCH - 1))

            # --- eviction + output DMA --------------------------------------
            stages = {}

            def evict(s, r):
                if s == S - 1 and r == NR - 1:
                    # tail: single-run transfer on SP (idle HWDGE, shortest
                    # chain after the last matmul)
                    st2 = oev.tile([128, C], f32, name="stz", tag="st")
                    nc.vector.tensor_scalar_add(st2[:], psb[(s, r)][:], 0.0)
                    nc.sync.dma_start(out_d[s, :, r, :], st2[:])
                    return
                if s == S - 1 and r == NR - 2:
                    st1 = oev.tile([128, C], f32, name="sty", tag="st")
                    nc.vector.tensor_scalar_add(st1[:], psb[(s, r)][:], 0.0)
                    nc.gpsimd.dma_start(out_d[s, :, r, :], st1[:])
                    return
                i = r // 2
                if r % 2 == 0:
                    stages[(s, i)] = oev.tile([128, 2 * C], f32,
                                              name=f"st_{s}_{i}", tag="st")
                dst = stages[(s, i)][:, (r % 2) * C : (r % 2 + 1) * C]
                nc.vector.tensor_scalar_add(dst, psb[(s, r)][:], 0.0)
                if r % 2 == 1:
                    st = stages[(s, i)][:].rearrange("p (b m) -> p b m", m=C)
                    nc.gpsimd.dma_start(out_d[s, :, 2 * i : 2 * i + 2, :], st)

            # --- sample 0: two waves matching DMA arrival -------------------
            for c in range(2):
                for r in range(5):
                    mm(0, r, c)
            for c in range(2, 4):
                for r in range(5):
                    mm(0, r, c)
            for c in range(4, NCH):
                for r in range(5):
                    mm(0, r, c)
            for r in range(5):
                evict(0, r)
            for r in range(5, NR):
                for c in range(NCH):
                    mm(0, r, c)
                evict(0, r)

            # --- sample 1: run-major ---------------------------------------
            for r in range(NR):
                for c in range(NCH):
                    mm(1, r, c)
                evict(1, r)

    nc.compile()
    return nc


def get_module():
    if "nc" not in _cache:
        _cache["nc"] = _build_module()
    return _cache["nc"]


def _route(x, gate_w, gate_b):
    """Replicates the reference router in numpy fp32. Returns combine [B,E]."""
    pooled = x.mean(axis=(2, 3), dtype=np.float32)
    logits = pooled @ gate_w + gate_b
    z = logits - logits.max(axis=-1, keepdims=True)
    ez = np.exp(z)
    w = ez / ez.sum(axis=-1, keepdims=True)
    topi = np.argsort(-w, axis=-1, kind="stable")[:, :TOPK]
    topw = np.take_along_axis(w, topi, axis=-1)
    topw = topw / (topw.sum(-1, keepdims=True) + 1e-10)
    combine = np.zeros((B, E), np.float32)
    np.put_along_axis(combine, topi, topw, axis=-1)
    return combine


_hp = {}  # host-prep cache for postprocess (border columns + bias)


def make_in_maps(x, gate_w, gate_b, expert_w, expert_b):
    import ml_dtypes

    bf16 = ml_dtypes.bfloat16
    x = np.ascontiguousarray(np.asarray(x, np.float32))
    gate_w = np.asarray(gate_w, np.float32)
    gate_b = np.asarray(gate_b, np.float32)
    expert_w = np.asarray(expert_w, np.float32)
    expert_b = np.asarray(expert_b, np.float32)

    combine = _route(x, gate_w, gate_b)                       # [B,E]
    Wc = np.einsum("be,eoikl->boikl", combine, expert_w)      # [B,C,C,3,3]
    bc = combine @ expert_b                                   # [B,C]

    # 32-wide flat image: rows 0/33 zero pad rows, no column padding
    xf = np.zeros((B, C, 34, RW), np.float32)
    xf[:, :, 1 : H + 1, :] = x
    xf = xf.reshape(B, C, FP)

    # Exact border columns (0 and 31) computed on host in fp32 — the device
    # result wraps across rows for horizontal taps there.
    xp34 = np.zeros((B, C, 34, 34), np.float32)
    xp34[:, :, 1 : H + 1, 1 : W + 1] = x
    border = np.zeros((B, C, H, 2), np.float32)
    for k in range(3):  # dy
        # col 0: input cols 0..2 of xp34; col 31: cols 30..32
        border[:, :, :, 0] += np.einsum(
            "boil,birl->bor", Wc[:, :, :, k, :], xp34[:, :, k : k + 32, 0:3]
        )
        border[:, :, :, 1] += np.einsum(
            "boil,birl->bor", Wc[:, :, :, k, :], xp34[:, :, k : k + 32, 31:34]
        )
    _hp["border"] = border + bc[:, :, None, None]
    _hp["bc"] = bc

    xfb = xf.astype(bf16)

    def with_margin(a):
        out = np.zeros(a.shape[:-1] + (TF,), bf16)
        out[..., MARG : MARG + FP] = a
        return out

    ta = with_margin(xfb[:, 0:128])
    img64 = xfb[:, 128:192]

    def dup_shift(d):
        t = np.zeros((B, 128, TF), bf16)
        t[:, 0:64, MARG : MARG + FP] = img64
        t[:, 64:128, MARG : MARG + FP - d] = img64[:, :, d:]
        return t

    tb32 = dup_shift(RW)
    tb1 = dup_shift(1)

    # Moving weights, one [<=128, 192] slab per K-chunk:
    #   WT[b, t, i, o] = Wc[b, o, i, dy, dx]
    WT = Wc.transpose(0, 3, 4, 2, 1).reshape(B, 9, C, C)
    w = np.zeros((B, 128, NCH * C), np.float32)
    for c in range(9):                        # ch 0-127, tap c
        w[:, :, c * C : (c + 1) * C] = WT[:, c, 0:128, :]
    for j in range(3):                        # pairs (0,j)+(1,j), d=32
        c = 9 + j
        w[:, 0:64, c * C : (c + 1) * C] = WT[:, j, 128:192, :]
        w[:, 64:128, c * C : (c + 1) * C] = WT[:, 3 + j, 128:192, :]
    w[:, 0:64, 12 * C : 13 * C] = WT[:, 6, 128:192, :]        # (2,0)+(2,1), d=1
    w[:, 64:128, 12 * C : 13 * C] = WT[:, 7, 128:192, :]
    w[:, 0:64, 13 * C : 14 * C] = WT[:, 8, 128:192, :]        # single (2,2)
    w = w.astype(bf16)

    taw = np.concatenate([w[:, :, 0 : 2 * C], ta, w[:, :, 2 * C : 4 * C]],
                         axis=-1)
    w = np.ascontiguousarray(w[:, :, 4 * C :])

    in_maps = []
    for cidx in range(NCORES):
        b0 = S * cidx
        in_maps.append(
            {
                "ta": np.ascontiguousarray(taw[b0 : b0 + S]),
                "tb32": np.ascontiguousarray(tb32[b0 : b0 + S]),
                "tb1": np.ascontiguousarray(tb1[b0 : b0 + S]),
                "w": np.ascontiguousarray(w[b0 : b0 + S]),
            }
        )
    return in_maps, bc


def postprocess(dev_out, bc_rows, border_rows):
    """[S, 128, NR, 192] device tensor -> [S,C,H,W], border cols replaced."""
    dev = np.asarray(dev_out, np.float32)
    # run r, position p -> flat 32 + 128r + p -> row 1 + (128r+p)//32
    grid = dev.transpose(0, 2, 1, 3).reshape(S, H, RW, C)  # rows 1..32
    out = grid.transpose(0, 3, 1, 2) + bc_rows[:, :, None, None]
    out[:, :, :, 0] = border_rows[:, :, :, 0]
    out[:, :, :, 31] = border_rows[:, :, :, 1]
    return out


def kernel(x, gate_w, gate_b, expert_w, expert_b):
    from concourse.bass_utils import run_bass_kernel_spmd

    nc = get_module()
    in_maps, bc = make_in_maps(x, gate_w, gate_b, expert_w, expert_b)
    border = _hp["border"]
    res = run_bass_kernel_spmd(nc, in_maps, core_ids=list(range(NCORES)))
    out = np.empty((B, C, H, W), np.float32)
    for c in range(NCORES):
        b0 = S * c
        out[b0 : b0 + S] = postprocess(res.results[c]["out"], bc[b0 : b0 + S],
                                       border[b0 : b0 + S])
    return out


# revision 33
# speedup vs baseline: 1.0166x; 1.0166x over previous
"""MoE block (B=16, C=192, H=W=32, E=8, top-2, 3x3 same-conv experts) on 8 trn2 cores.

Strategy (v6, compact flat-run pixel-stationary):
  - Router + top-2 combine computed on host; conv linearity folds the
    expert mix into ONE conv per sample (combined weights). 2 convs/core.
  - Matmul formulation: out[M=pixels, N=192 out-ch], pixel positions
    STATIONARY, weights MOVING. Streamed rows per matmul = 192 (vs 512
    for the out-ch-stationary form).
  - Walrus requires the stationary AP to have a single free dimension,
    so the image is stored 32-wide (no column padding, only top/bottom
    pad rows): real outputs are flat [32, 1056) = exactly 8 contiguous
    128-runs, and a 3x3 tap is a uniform flat shift s = (dy-1)*32+(dx-1)
    of the run. Horizontal taps wrap across row boundaries, corrupting
    output columns 0 and 31 — the host recomputes those two columns
    exactly (tiny einsum) and overwrites them.
  - Contraction (9 taps x 192 ch = 1728) in 14 K-chunks per run: 9 full
    chunks for ch 0-127 (one per tap, plain image tile Ta), and
    ch 128-191 packed two-taps-per-chunk using duplicate tiles whose
    upper 64 partitions hold the same channels pre-shifted by the
    inter-tap flat offset (d=32 vertical pairs, d=1 horizontal), so one
    AP covers both tap halves: 4 paired chunks + 1 single K=64 chunk.
  - 2 samples x 8 runs x 14 chunks = 224 matmuls of 192 rows — the
    128x128 PE packing floor for this conv in bf16.
  - PE kept continuously busy from ~1.1us with warmup matmuls (an idle
    PE gap resets the p-state ramp).
  - Input DMAs on SP (HWDGE) + Pool (SWDGE) in consumption order;
    output DMAs on Pool; the DMA-completion semaphore fires ~900ns
    after the transfer, so everything is prefetched one phase ahead.
"""

import numpy as np

B, C, H, W = 16, 192, 32, 32
E, TOPK = 8, 2
NCORES = 8
S = B // NCORES          # samples per core
RW = 32                  # row width (no column padding)
FP = 34 * RW             # flat image size 1088 (rows 0/33 are pad rows)
MARG = 33                # flat-shift margin
TF = FP + 2 * MARG       # tile free size 1154
NR = 8                   # M-runs per sample (8 x 128)
RUNS = [(RW + 128 * i, 128) for i in range(NR)]
NCH = 14                 # K-chunks per run
TAPS = [(t // 3, t % 3) for t in range(9)]

# Warmup matmul row-counts: first runs at the LOW p-state, rest at MID.
WARMUP_NS = [192] * 14

_cache = {}


def _shift(t):
    dy, dx = TAPS[t]
    return (dy - 1) * RW + (dx - 1)


def _build_module():
    import concourse.tile as tile
    from concourse import bacc, mybir

    f32 = mybir.dt.float32
    bf16 = mybir.dt.bfloat16

    nc = bacc.Bacc("TRN2", target_bir_lowering=False, debug=False, num_devices=NCORES)
    ta_d = nc.dram_tensor("ta", [S, 128, TF], bf16, kind="ExternalInput")
    tb32_d = nc.dram_tensor("tb32", [S, 128, TF], bf16, kind="ExternalInput")
    tb1_d = nc.dram_tensor("tb1", [S, 128, TF], bf16, kind="ExternalInput")
    w_d = nc.dram_tensor("w", [S, 128, NCH * C], bf16, kind="ExternalInput")
    out_d = nc.dram_tensor("out", [S, 128, NR, C], f32, kind="ExternalOutput")

    with tile.TileContext(nc) as tc:
        with (
            tc.tile_pool(name="img", bufs=1) as img,
            tc.tile_pool(name="win", bufs=1) as win,
            tc.tile_pool(name="cst", bufs=1) as cst,
            tc.tile_pool(name="ps", bufs=8, space="PSUM") as ps,
            tc.tile_pool(name="oev", bufs=4) as oev,
        ):
            Ta, T32, T1, Wt = {}, {}, {}, {}

            wt0 = win.tile([128, NCH * C], bf16, name="W_0", tag="W_0")
            Wt[0] = wt0
            wt1 = win.tile([128, NCH * C], bf16, name="W_1", tag="W_1")
            Wt[1] = wt1
            for s, nm in ((0, "Ta_0"), (1, "Ta_1")):
                Ta[s] = img.tile([128, TF], bf16, name=nm, tag=nm)
            for s, nm in ((0, "T32_0"), (1, "T32_1")):
                T32[s] = img.tile([128, TF], bf16, name=nm, tag=nm)
            for s, nm in ((0, "T1_0"), (1, "T1_1")):
                T1[s] = img.tile([128, TF], bf16, name=nm, tag=nm)

            # --- input DMAs: SP (HWDGE) + Pool (SWDGE), consumption order ---
            # First two transfers (w chunks 0-1 via Pool, Ta runs 0-2 prefix
            # via SP) use both DGE paths in parallel for the earliest start.
            R_A0 = MARG + RW + 128 * 3 + MARG  # Ta prefix for runs 0-2
            R_A = MARG + RW + 128 * 5 + MARG   # ... extended to runs 3-4
            nc.gpsimd.dma_start(wt0[:, 0 : 2 * C], w_d[0, :, 0 : 2 * C])
            nc.sync.dma_start(Ta[0][:, 0:R_A0], ta_d[0, :, 0:R_A0])
            nc.sync.dma_start(Ta[0][:, R_A0:R_A], ta_d[0, :, R_A0:R_A])
            nc.gpsimd.dma_start(wt0[:, 4 * C : 9 * C], w_d[0, :, 4 * C : 9 * C])
            nc.sync.dma_start(wt0[:, 2 * C : 4 * C], w_d[0, :, 2 * C : 4 * C])
            nc.sync.dma_start(T32[0][:], tb32_d[0])
            nc.gpsimd.dma_start(Ta[0][:, R_A:TF], ta_d[0, :, R_A:TF])
            nc.sync.dma_start(wt0[:, 9 * C : NCH * C], w_d[0, :, 9 * C : NCH * C])
            nc.sync.dma_start(T1[0][:], tb1_d[0])
            nc.gpsimd.dma_start(Ta[1][:], ta_d[1])
            nc.sync.dma_start(wt1[:, 0 : 9 * C], w_d[1, :, 0 : 9 * C])
            nc.sync.dma_start(T32[1][:], tb32_d[1])
            nc.sync.dma_start(wt1[:, 9 * C : NCH * C], w_d[1, :, 9 * C : NCH * C])
            nc.sync.dma_start(T1[1][:], tb1_d[1])

            # --- PSUM tiles (8 banks, cycled) -------------------------------
            psb = {}
            for s in range(S):
                for r in range(NR):
                    psb[(s, r)] = ps.tile([128, C], f32, name=f"ps_{s}_{r}",
                                          tag="ps")

            # --- PE warmup on zeros -----------------------------------------
            scr = cst.tile([128, C], bf16, name="scr", tag="scr")
            nc.vector.memset(scr[:], 0.0)
            for n in WARMUP_NS:
                nc.tensor.matmul(psb[(0, 0)][:, 0:n], scr[:, 0:128], scr[:, 0:n],
                                 start=True, stop=True, skip_group_check=True)

            # --- matmul emission --------------------------------------------
            def lhsT(s, r, c):
                base, ln = RUNS[r]
                if c < 9:  # ch 0-127, tap c
                    st = MARG + base + _shift(c)
                    return Ta[s][:, st : st + ln]
                if c < 12:  # pairs (0,j)+(1,j), d=32 baked into T32's upper
                    st = MARG + base + _shift(c - 9)
                    return T32[s][:, st : st + ln]
                if c == 12:  # pair (2,0)+(2,1), d=1 baked into T1's upper
                    st = MARG + base + _shift(6)
                    return T1[s][:, st : st + ln]
                # single (2,2), K=64 on T1's lower half
                st = MARG + base + _shift(8)
                return T1[s][0:64, st : st + ln]

            def rhs(s, c):
                if c == NCH - 1:
                    return Wt[s][0:64, c * C : (c + 1) * C]
                return Wt[s][:, c * C : (c + 1) * C]

            def mm(s, r, c):
                nc.tensor.matmul(psb[(s, r)][:], lhsT(s, r, c), rhs(s, c),
                                 start=(c == 0), stop=(c == NCH - 1))

            # --- eviction + output DMA --------------------------------------
            stages = {}

            def evict(s, r):
                if s == S - 1 and r == NR - 1:
                    # tail: single-run transfer on SP (idle HWDGE, shortest
                    # chain after the last matmul)
                    st2 = oev.tile([128, C], f32, name="stz", tag="st")
                    nc.vector.tensor_scalar_add(st2[:], psb[(s, r)][:], 0.0)
                    nc.sync.dma_start(out_d[s, :, r, :], st2[:])
                    return
                if s == S - 1 and r == NR - 2:
                    st1 = oev.tile([128, C], f32, name="sty", tag="st")
                    nc.vector.tensor_scalar_add(st1[:], psb[(s, r)][:], 0.0)
                    nc.gpsimd.dma_start(out_d[s, :, r, :], st1[:])
                    return
                i = r // 2
                if r % 2 == 0:
                    stages[(s, i)] = oev.tile([128, 2 * C], f32,
                                              name=f"st_{s}_{i}", tag="st")
                dst = stages[(s, i)][:, (r % 2) * C : (r % 2 + 1) * C]
                nc.vector.tensor_scalar_add(dst, psb[(s, r)][:], 0.0)
                if r % 2 == 1:
                    st = stages[(s, i)][:].rearrange("p (b m) -> p b m", m=C)
                    nc.gpsimd.dma_start(out_d[s, :, 2 * i : 2 * i + 2, :], st)

            # --- sample 0: two waves matching DMA arrival -------------------
            for c in range(2):
                for r in range(5):
                    mm(0, r, c)
            # ~107ns of junk fill: c2-c3 weights land one HWDGE slot later
            for _ in range(2):
                nc.tensor.matmul(psb[(1, 0)][:, 0:128], scr[:, 0:128],
                                 scr[:, 0:128], start=True, stop=True,
                                 skip_group_check=True)
            for c in range(2, 4):
                for r in range(5):
                    mm(0, r, c)
            for c in range(4, NCH):
                for r in range(5):
                    mm(0, r, c)
            for r in range(5):
                evict(0, r)
            for r in range(5, NR):
                for c in range(NCH):
                    mm(0, r, c)
                evict(0, r)

            # --- sample 1: run-major ---------------------------------------
            for r in range(NR):
                for c in range(NCH):
                    mm(1, r, c)
                evict(1, r)

    nc.compile()
    return nc


def get_module():
    if "nc" not in _cache:
        _cache["nc"] = _build_module()
    return _cache["nc"]


def _route(x, gate_w, gate_b):
    """Replicates the reference router in numpy fp32. Returns combine [B,E]."""
    pooled = x.mean(axis=(2, 3), dtype=np.float32)
    logits = pooled @ gate_w + gate_b
    z = logits - logits.max(axis=-1, keepdims=True)
    ez = np.exp(z)
    w = ez / ez.sum(axis=-1, keepdims=True)
    topi = np.argsort(-w, axis=-1, kind="stable")[:, :TOPK]
    topw = np.take_along_axis(w, topi, axis=-1)
    topw = topw / (topw.sum(-1, keepdims=True) + 1e-10)
    combine = np.zeros((B, E), np.float32)
    np.put_along_axis(combine, topi, topw, axis=-1)
    return combine


_hp = {}  # host-prep cache for postprocess (border columns + bias)


def make_in_maps(x, gate_w, gate_b, expert_w, expert_b):
    import ml_dtypes

    bf16 = ml_dtypes.bfloat16
    x = np.ascontiguousarray(np.asarray(x, np.float32))
    gate_w = np.asarray(gate_w, np.float32)
    gate_b = np.asarray(gate_b, np.float32)
    expert_w = np.asarray(expert_w, np.float32)
    expert_b = np.asarray(expert_b, np.float32)

    combine = _route(x, gate_w, gate_b)                       # [B,E]
    Wc = np.einsum("be,eoikl->boikl", combine, expert_w)      # [B,C,C,3,3]
    bc = combine @ expert_b                                   # [B,C]

    # 32-wide flat image: rows 0/33 zero pad rows, no column padding
    xf = np.zeros((B, C, 34, RW), np.float32)
    xf[:, :, 1 : H + 1, :] = x
    xf = xf.reshape(B, C, FP)

    # Exact border columns (0 and 31) computed on host in fp32 — the device
    # result wraps across rows for horizontal taps there.
    xp34 = np.zeros((B, C, 34, 34), np.float32)
    xp34[:, :, 1 : H + 1, 1 : W + 1] = x
    border = np.zeros((B, C, H, 2), np.float32)
    for k in range(3):  # dy
        # col 0: input cols 0..2 of xp34; col 31: cols 30..32
        border[:, :, :, 0] += np.einsum(
            "boil,birl->bor", Wc[:, :, :, k, :], xp34[:, :, k : k + 32, 0:3]
        )
        border[:, :, :, 1] += np.einsum(
            "boil,birl->bor", Wc[:, :, :, k, :], xp34[:, :, k : k + 32, 31:34]
        )
    _hp["border"] = border + bc[:, :, None, None]
    _hp["bc"] = bc

    xfb = xf.astype(bf16)

    def with_margin(a):
        out = np.zeros(a.shape[:-1] + (TF,), bf16)
        out[..., MARG : MARG + FP] = a
        return out

    ta = with_margin(xfb[:, 0:128])
    img64 = xfb[:, 128:192]

    def dup_shift(d):
        t = np.zeros((B, 128, TF), bf16)
        t[:, 0:64, MARG : MARG + FP] = img64
        t[:, 64:128, MARG : MARG + FP - d] = img64[:, :, d:]
        return t

    tb32 = dup_shift(RW)
    tb1 = dup_shift(1)

    # Moving weights, one [<=128, 192] slab per K-chunk:
    #   WT[b, t, i, o] = Wc[b, o, i, dy, dx]
    WT = Wc.transpose(0, 3, 4, 2, 1).reshape(B, 9, C, C)
    w = np.zeros((B, 128, NCH * C), np.float32)
    for c in range(9):                        # ch 0-127, tap c
        w[:, :, c * C : (c + 1) * C] = WT[:, c, 0:128, :]
    for j in range(3):                        # pairs (0,j)+(1,j), d=32
        c = 9 + j
        w[:, 0:64, c * C : (c + 1) * C] = WT[:, j, 128:192, :]
        w[:, 64:128, c * C : (c + 1) * C] = WT[:, 3 + j, 128:192, :]
    w[:, 0:64, 12 * C : 13 * C] = WT[:, 6, 128:192, :]        # (2,0)+(2,1), d=1
    w[:, 64:128, 12 * C : 13 * C] = WT[:, 7, 128:192, :]
    w[:, 0:64, 13 * C : 14 * C] = WT[:, 8, 128:192, :]        # single (2,2)
    w = w.astype(bf16)

    in_maps = []
    for cidx in range(NCORES):
        b0 = S * cidx
        in_maps.append(
            {
                "ta": np.ascontiguousarray(ta[b0 : b0 + S]),
                "tb32": np.ascontiguousarray(tb32[b0 : b0 + S]),
                "tb1": np.ascontiguousarray(tb1[b0 : b0 + S]),
                "w": np.ascontiguousarray(w[b0 : b0 + S]),
            }
        )
    return in_maps, bc


def postprocess(dev_out, bc_rows, border_rows):
    """[S, 128, NR, 192] device tensor -> [S,C,H,W], border cols replaced."""
    dev = np.asarray(dev_out, np.float32)
    # run r, position p -> flat 32 + 128r + p -> row 1 + (128r+p)//32
    grid = dev.transpose(0, 2, 1, 3).reshape(S, H, RW, C)  # rows 1..32
    out = grid.transpose(0, 3, 1, 2) + bc_rows[:, :, None, None]
    out[:, :, :, 0] = border_rows[:, :, :, 0]
    out[:, :, :, 31] = border_rows[:, :, :, 1]
    return out


def kernel(x, gate_w, gate_b, expert_w, expert_b):
    from concourse.bass_utils import run_bass_kernel_spmd

    nc = get_module()
    in_maps, bc = make_in_maps(x, gate_w, gate_b, expert_w, expert_b)
    border = _hp["border"]
    res = run_bass_kernel_spmd(nc, in_maps, core_ids=list(range(NCORES)))
    out = np.empty((B, C, H, W), np.float32)
    for c in range(NCORES):
        b0 = S * c
        out[b0 : b0 + S] = postprocess(res.results[c]["out"], bc[b0 : b0 + S],
                                       border[b0 : b0 + S])
    return out


# revision 34
# speedup vs baseline: 1.0208x; 1.0042x over previous
"""MoE block (B=16, C=192, H=W=32, E=8, top-2, 3x3 same-conv experts) on 8 trn2 cores.

Strategy (v6, compact flat-run pixel-stationary):
  - Router + top-2 combine computed on host; conv linearity folds the
    expert mix into ONE conv per sample (combined weights). 2 convs/core.
  - Matmul formulation: out[M=pixels, N=192 out-ch], pixel positions
    STATIONARY, weights MOVING. Streamed rows per matmul = 192 (vs 512
    for the out-ch-stationary form).
  - Walrus requires the stationary AP to have a single free dimension,
    so the image is stored 32-wide (no column padding, only top/bottom
    pad rows): real outputs are flat [32, 1056) = exactly 8 contiguous
    128-runs, and a 3x3 tap is a uniform flat shift s = (dy-1)*32+(dx-1)
    of the run. Horizontal taps wrap across row boundaries, corrupting
    output columns 0 and 31 — the host recomputes those two columns
    exactly (tiny einsum) and overwrites them.
  - Contraction (9 taps x 192 ch = 1728) in 14 K-chunks per run: 9 full
    chunks for ch 0-127 (one per tap, plain image tile Ta), and
    ch 128-191 packed two-taps-per-chunk using duplicate tiles whose
    upper 64 partitions hold the same channels pre-shifted by the
    inter-tap flat offset (d=32 vertical pairs, d=1 horizontal), so one
    AP covers both tap halves: 4 paired chunks + 1 single K=64 chunk.
  - 2 samples x 8 runs x 14 chunks = 224 matmuls of 192 rows — the
    128x128 PE packing floor for this conv in bf16.
  - PE kept continuously busy from ~1.1us with warmup matmuls (an idle
    PE gap resets the p-state ramp).
  - Input DMAs on SP (HWDGE) + Pool (SWDGE) in consumption order;
    output DMAs on Pool; the DMA-completion semaphore fires ~900ns
    after the transfer, so everything is prefetched one phase ahead.
"""

import numpy as np

B, C, H, W = 16, 192, 32, 32
E, TOPK = 8, 2
NCORES = 8
S = B // NCORES          # samples per core
RW = 32                  # row width (no column padding)
FP = 34 * RW             # flat image size 1088 (rows 0/33 are pad rows)
MARG = 33                # flat-shift margin
TF = FP + 2 * MARG       # tile free size 1154
NR = 8                   # M-runs per sample (8 x 128)
RUNS = [(RW + 128 * i, 128) for i in range(NR)]
NCH = 14                 # K-chunks per run
TAPS = [(t // 3, t % 3) for t in range(9)]

# Warmup matmul row-counts: first runs at the LOW p-state, rest at MID.
WARMUP_NS = [192] * 14

_cache = {}


def _shift(t):
    dy, dx = TAPS[t]
    return (dy - 1) * RW + (dx - 1)


def _build_module():
    import concourse.tile as tile
    from concourse import bacc, mybir

    f32 = mybir.dt.float32
    bf16 = mybir.dt.bfloat16

    nc = bacc.Bacc("TRN2", target_bir_lowering=False, debug=False, num_devices=NCORES)
    ta_d = nc.dram_tensor("ta", [S, 128, TF], bf16, kind="ExternalInput")
    tb32_d = nc.dram_tensor("tb32", [S, 128, TF], bf16, kind="ExternalInput")
    tb1_d = nc.dram_tensor("tb1", [S, 128, TF], bf16, kind="ExternalInput")
    w_d = nc.dram_tensor("w", [S, 128, NCH * C], bf16, kind="ExternalInput")
    out_d = nc.dram_tensor("out", [S, 128, NR, C], f32, kind="ExternalOutput")

    with tile.TileContext(nc) as tc:
        with (
            tc.tile_pool(name="img", bufs=1) as img,
            tc.tile_pool(name="win", bufs=1) as win,
            tc.tile_pool(name="cst", bufs=1) as cst,
            tc.tile_pool(name="ps", bufs=8, space="PSUM") as ps,
            tc.tile_pool(name="oev", bufs=4) as oev,
        ):
            Ta, T32, T1, Wt = {}, {}, {}, {}

            wt0 = win.tile([128, NCH * C], bf16, name="W_0", tag="W_0")
            Wt[0] = wt0
            wt1 = win.tile([128, NCH * C], bf16, name="W_1", tag="W_1")
            Wt[1] = wt1
            for s, nm in ((0, "Ta_0"), (1, "Ta_1")):
                Ta[s] = img.tile([128, TF], bf16, name=nm, tag=nm)
            for s, nm in ((0, "T32_0"), (1, "T32_1")):
                T32[s] = img.tile([128, TF], bf16, name=nm, tag=nm)
            for s, nm in ((0, "T1_0"), (1, "T1_1")):
                T1[s] = img.tile([128, TF], bf16, name=nm, tag=nm)

            # --- input DMAs: SP (HWDGE) + Pool (SWDGE), consumption order ---
            # First two transfers (w chunks 0-1 via Pool, Ta runs 0-2 prefix
            # via SP) use both DGE paths in parallel for the earliest start.
            R_A0 = MARG + RW + 128 * 3 + MARG  # Ta prefix for runs 0-2
            R_A = MARG + RW + 128 * 5 + MARG   # ... extended to runs 3-4
            nc.gpsimd.dma_start(wt0[:, 0 : 2 * C], w_d[0, :, 0 : 2 * C])
            nc.sync.dma_start(Ta[0][:, 0:R_A0], ta_d[0, :, 0:R_A0])
            nc.sync.dma_start(Ta[0][:, R_A0:R_A], ta_d[0, :, R_A0:R_A])
            nc.gpsimd.dma_start(wt0[:, 4 * C : 9 * C], w_d[0, :, 4 * C : 9 * C])
            nc.sync.dma_start(wt0[:, 2 * C : 4 * C], w_d[0, :, 2 * C : 4 * C])
            nc.sync.dma_start(T32[0][:], tb32_d[0])
            nc.gpsimd.dma_start(Ta[0][:, R_A:TF], ta_d[0, :, R_A:TF])
            nc.sync.dma_start(wt0[:, 9 * C : NCH * C], w_d[0, :, 9 * C : NCH * C])
            nc.sync.dma_start(T1[0][:], tb1_d[0])
            nc.gpsimd.dma_start(Ta[1][:], ta_d[1])
            nc.sync.dma_start(wt1[:, 0 : 9 * C], w_d[1, :, 0 : 9 * C])
            nc.sync.dma_start(T32[1][:], tb32_d[1])
            nc.sync.dma_start(wt1[:, 9 * C : NCH * C], w_d[1, :, 9 * C : NCH * C])
            nc.sync.dma_start(T1[1][:], tb1_d[1])

            # --- PSUM tiles (8 banks, cycled) -------------------------------
            psb = {}
            for s in range(S):
                for r in range(NR):
                    psb[(s, r)] = ps.tile([128, C], f32, name=f"ps_{s}_{r}",
                                          tag="ps")

            # --- PE warmup on zeros -----------------------------------------
            scr = cst.tile([128, C], bf16, name="scr", tag="scr")
            nc.vector.memset(scr[:], 0.0)
            for n in WARMUP_NS:
                nc.tensor.matmul(psb[(0, 0)][:, 0:n], scr[:, 0:128], scr[:, 0:n],
                                 start=True, stop=True, skip_group_check=True)

            # --- matmul emission --------------------------------------------
            def lhsT(s, r, c):
                base, ln = RUNS[r]
                if c < 9:  # ch 0-127, tap c
                    st = MARG + base + _shift(c)
                    return Ta[s][:, st : st + ln]
                if c < 12:  # pairs (0,j)+(1,j), d=32 baked into T32's upper
                    st = MARG + base + _shift(c - 9)
                    return T32[s][:, st : st + ln]
                if c == 12:  # pair (2,0)+(2,1), d=1 baked into T1's upper
                    st = MARG + base + _shift(6)
                    return T1[s][:, st : st + ln]
                # single (2,2), K=64 on T1's lower half
                st = MARG + base + _shift(8)
                return T1[s][0:64, st : st + ln]

            def rhs(s, c):
                if c == NCH - 1:
                    return Wt[s][0:64, c * C : (c + 1) * C]
                return Wt[s][:, c * C : (c + 1) * C]

            def mm(s, r, c):
                nc.tensor.matmul(psb[(s, r)][:], lhsT(s, r, c), rhs(s, c),
                                 start=(c == 0), stop=(c == NCH - 1))

            # --- eviction + output DMA --------------------------------------
            stages = {}

            def evict(s, r):
                if s == S - 1 and r == NR - 1:
                    # tail: single-run transfer on SP (idle HWDGE, shortest
                    # chain after the last matmul)
                    st2 = oev.tile([128, C], f32, name="stz", tag="st")
                    nc.vector.tensor_scalar_add(st2[:], psb[(s, r)][:], 0.0)
                    nc.sync.dma_start(out_d[s, :, r, :], st2[:])
                    return
                if s == S - 1 and r == NR - 2:
                    st1 = oev.tile([128, C], f32, name="sty", tag="st")
                    nc.vector.tensor_scalar_add(st1[:], psb[(s, r)][:], 0.0)
                    nc.gpsimd.dma_start(out_d[s, :, r, :], st1[:])
                    return
                i = r // 2
                if r % 2 == 0:
                    stages[(s, i)] = oev.tile([128, 2 * C], f32,
                                              name=f"st_{s}_{i}", tag="st")
                dst = stages[(s, i)][:, (r % 2) * C : (r % 2 + 1) * C]
                nc.vector.tensor_scalar_add(dst, psb[(s, r)][:], 0.0)
                if r % 2 == 1:
                    st = stages[(s, i)][:].rearrange("p (b m) -> p b m", m=C)
                    nc.gpsimd.dma_start(out_d[s, :, 2 * i : 2 * i + 2, :], st)

            # --- sample 0: two waves matching DMA arrival -------------------
            for c in range(2):
                for r in range(5):
                    mm(0, r, c)
            for c in range(2, 4):
                for r in range(5):
                    mm(0, r, c)
            for c in range(4, NCH):
                for r in range(5):
                    mm(0, r, c)
            for r in range(5):
                evict(0, r)
            for r in range(5, NR):
                for c in range(NCH):
                    mm(0, r, c)
                evict(0, r)

            # --- sample 1: run-major ---------------------------------------
            for r in range(NR):
                for c in range(NCH):
                    mm(1, r, c)
                evict(1, r)

    nc.compile()
    return nc


def get_module():
    if "nc" not in _cache:
        _cache["nc"] = _build_module()
    return _cache["nc"]


def _route(x, gate_w, gate_b):
    """Replicates the reference router in numpy fp32. Returns combine [B,E]."""
    pooled = x.mean(axis=(2, 3), dtype=np.float32)
    logits = pooled @ gate_w + gate_b
    z = logits - logits.max(axis=-1, keepdims=True)
    ez = np.exp(z)
    w = ez / ez.sum(axis=-1, keepdims=True)
    topi = np.argsort(-w, axis=-1, kind="stable")[:, :TOPK]
    topw = np.take_along_axis(w, topi, axis=-1)
    topw = topw / (topw.sum(-1, keepdims=True) + 1e-10)
    combine = np.zeros((B, E), np.float32)
    np.put_along_axis(combine, topi, topw, axis=-1)
    return combine


_hp = {}  # host-prep cache for postprocess (border columns + bias)


def make_in_maps(x, gate_w, gate_b, expert_w, expert_b):
    import ml_dtypes

    bf16 = ml_dtypes.bfloat16
    x = np.ascontiguousarray(np.asarray(x, np.float32))
    gate_w = np.asarray(gate_w, np.float32)
    gate_b = np.asarray(gate_b, np.float32)
    expert_w = np.asarray(expert_w, np.float32)
    expert_b = np.asarray(expert_b, np.float32)

    combine = _route(x, gate_w, gate_b)                       # [B,E]
    Wc = np.einsum("be,eoikl->boikl", combine, expert_w)      # [B,C,C,3,3]
    bc = combine @ expert_b                                   # [B,C]

    # 32-wide flat image: rows 0/33 zero pad rows, no column padding
    xf = np.zeros((B, C, 34, RW), np.float32)
    xf[:, :, 1 : H + 1, :] = x
    xf = xf.reshape(B, C, FP)

    # Exact border columns (0 and 31) computed on host in fp32 — the device
    # result wraps across rows for horizontal taps there.
    xp34 = np.zeros((B, C, 34, 34), np.float32)
    xp34[:, :, 1 : H + 1, 1 : W + 1] = x
    border = np.zeros((B, C, H, 2), np.float32)
    for k in range(3):  # dy
        # col 0: input cols 0..2 of xp34; col 31: cols 30..32
        border[:, :, :, 0] += np.einsum(
            "boil,birl->bor", Wc[:, :, :, k, :], xp34[:, :, k : k + 32, 0:3]
        )
        border[:, :, :, 1] += np.einsum(
            "boil,birl->bor", Wc[:, :, :, k, :], xp34[:, :, k : k + 32, 31:34]
        )
    _hp["border"] = border + bc[:, :, None, None]
    _hp["bc"] = bc

    xfb = xf.astype(bf16)

    def with_margin(a):
        out = np.zeros(a.shape[:-1] + (TF,), bf16)
        out[..., MARG : MARG + FP] = a
        return out

    ta = with_margin(xfb[:, 0:128])
    img64 = xfb[:, 128:192]

    def dup_shift(d):
        t = np.zeros((B, 128, TF), bf16)
        t[:, 0:64, MARG : MARG + FP] = img64
        t[:, 64:128, MARG : MARG + FP - d] = img64[:, :, d:]
        return t

    tb32 = dup_shift(RW)
    tb1 = dup_shift(1)

    # Moving weights, one [<=128, 192] slab per K-chunk:
    #   WT[b, t, i, o] = Wc[b, o, i, dy, dx]
    WT = Wc.transpose(0, 3, 4, 2, 1).reshape(B, 9, C, C)
    w = np.zeros((B, 128, NCH * C), np.float32)
    for c in range(9):                        # ch 0-127, tap c
        w[:, :, c * C : (c + 1) * C] = WT[:, c, 0:128, :]
    for j in range(3):                        # pairs (0,j)+(1,j), d=32
        c = 9 + j
        w[:, 0:64, c * C : (c + 1) * C] = WT[:, j, 128:192, :]
        w[:, 64:128, c * C : (c + 1) * C] = WT[:, 3 + j, 128:192, :]
    w[:, 0:64, 12 * C : 13 * C] = WT[:, 6, 128:192, :]        # (2,0)+(2,1), d=1
    w[:, 64:128, 12 * C : 13 * C] = WT[:, 7, 128:192, :]
    w[:, 0:64, 13 * C : 14 * C] = WT[:, 8, 128:192, :]        # single (2,2)
    w = w.astype(bf16)

    in_maps = []
    for cidx in range(NCORES):
        b0 = S * cidx
        in_maps.append(
            {
                "ta": np.ascontiguousarray(ta[b0 : b0 + S]),
                "tb32": np.ascontiguousarray(tb32[b0 : b0 + S]),
                "tb1": np.ascontiguousarray(tb1[b0 : b0 + S]),
                "w": np.ascontiguousarray(w[b0 : b0 + S]),
            }
        )
    return in_maps, bc


def postprocess(dev_out, bc_rows, border_rows):
    """[S, 128, NR, 192] device tensor -> [S,C,H,W], border cols replaced."""
    dev = np.asarray(dev_out, np.float32)
    # run r, position p -> flat 32 + 128r + p -> row 1 + (128r+p)//32
    grid = dev.transpose(0, 2, 1, 3).reshape(S, H, RW, C)  # rows 1..32
    out = grid.transpose(0, 3, 1, 2) + bc_rows[:, :, None, None]
    out[:, :, :, 0] = border_rows[:, :, :, 0]
    out[:, :, :, 31] = border_rows[:, :, :, 1]
    return out


def kernel(x, gate_w, gate_b, expert_w, expert_b):
    from concourse.bass_utils import run_bass_kernel_spmd

    nc = get_module()
    in_maps, bc = make_in_maps(x, gate_w, gate_b, expert_w, expert_b)
    border = _hp["border"]
    res = run_bass_kernel_spmd(nc, in_maps, core_ids=list(range(NCORES)))
    out = np.empty((B, C, H, W), np.float32)
    for c in range(NCORES):
        b0 = S * c
        out[b0 : b0 + S] = postprocess(res.results[c]["out"], bc[b0 : b0 + S],
                                       border[b0 : b0 + S])
    return out
